# revision 17
# baseline (speedup 1.0000x reference)
"""Causal self-attention kernel for 8 TRN2 NeuronCores.

Problem: B=2, T=2048, C=1024, H=16, DH=64, fp32.
Sharding: tensor-parallel over heads (2 heads/core, both batches computed on
every core), then an 8-core AllToAll redistributes attention outputs so each
core owns a 512-row slice of the flattened (B*T, C) output for the final
projection (sequence-parallel output projection, no reduction needed).

Per core:
  1. QKV projections for its 2 heads (both batches), q/k kept transposed
     (head-dim on partitions), v transposed back to natural layout via PE
     transpose with a fused ones-column for row-sum accumulation.
  2. Causal flash-style attention without max-subtraction (logits are ~N(0,1),
     exp never overflows in fp32): S^T blocks -> exp on ScalarE -> P^T @ v
     accumulated in PSUM with an extra "ones" row computing the softmax
     denominator in the same matmuls.
  3. Normalize, stage to DRAM, AllToAll, then out-slice = yT_slice.T @ Wproj.T
     + bias.

Matmuls run in float32r (full-rate fp32, ~1.5e-4 relative rounding).
"""

import sys

sys.path.insert(0, "/opt/trn_rl_repo")

from contextlib import ExitStack

import numpy as np

import concourse.bass as bass
import concourse.mybir as mybir
import concourse.tile as tile
from concourse import bacc
from concourse.bass_utils import run_bass_kernel_spmd
from concourse.masks import make_identity

F32 = mybir.dt.float32
F32R = mybir.dt.float32r
EXP = mybir.ActivationFunctionType.Exp

B, T, C, H, DH = 2, 2048, 1024, 16, 64
N_CORES = 8
HPC = H // N_CORES          # heads per core = 2
P = 128
TQ = 512                    # q window (matmul free dim)
NQ = T // TQ                # q chunks per batch = 4
TS = 128                    # s chunk (contraction tile for P^T @ v)
NC_CHUNKS = C // P          # 8 contraction chunks for projections
SCALE = 1.0 / np.sqrt(DH)

_CACHED = {}


def _build():
    nc = bacc.Bacc("TRN2", target_bir_lowering=False, debug=False,
                   num_devices=N_CORES)

    xT = [nc.dram_tensor(f"xT{b}", [C, T], F32R, kind="ExternalInput").ap()
          for b in range(B)]
    wq = nc.dram_tensor("wq", [C, HPC * DH], F32R, kind="ExternalInput").ap()
    wk = nc.dram_tensor("wk", [C, HPC * DH], F32R, kind="ExternalInput").ap()
    wv = nc.dram_tensor("wv", [C, HPC * DH], F32R, kind="ExternalInput").ap()
    wpT = nc.dram_tensor("wpT", [C, C], F32R, kind="ExternalInput").ap()
    # diag-block masks: maskA = [m0|m1], maskB = [m2|m3];
    # m_m[r, c] = 1.0 if c >= r + 128*m else 0.0   (within a 128x512 block)
    maskA = nc.dram_tensor("maskA", [P, 2 * TQ], F32R, kind="ExternalInput").ap()
    maskB = nc.dram_tensor("maskB", [P, 2 * TQ], F32R, kind="ExternalInput").ap()
    out = nc.dram_tensor("out", [TQ, C], F32, kind="ExternalOutput").ap()

    x_a2a = nc.dram_tensor("x_a2a", [N_CORES * P, TQ], F32R)
    o_a2a = nc.dram_tensor("o_a2a", [N_CORES * P, TQ], F32R)

    with tile.TileContext(nc) as tc, ExitStack() as ctx:
        # ---- persistent pools ----
        pool_const = ctx.enter_context(tc.tile_pool(name="const", bufs=1))
        pool_x = ctx.enter_context(tc.tile_pool(name="xpool", bufs=NC_CHUNKS))
        pool_qk = ctx.enter_context(tc.tile_pool(name="qkpool", bufs=2))
        pool_vaug = ctx.enter_context(tc.tile_pool(name="vaugpool", bufs=32))
        pool_vt = ctx.enter_context(tc.tile_pool(name="vtpool", bufs=2))
        pool_pt = ctx.enter_context(tc.tile_pool(name="ptpool", bufs=3))
        pool_ytn = ctx.enter_context(tc.tile_pool(name="ytnpool", bufs=2))
        pool_o = ctx.enter_context(tc.tile_pool(name="opool", bufs=3))
        pool_outsb = ctx.enter_context(tc.tile_pool(name="outsbpool", bufs=3))

        identity = pool_const.tile([P, P], F32, tag="ident")
        make_identity(nc, identity[:])

        wq_sb = pool_const.tile([P, NC_CHUNKS * P], F32R, tag="wq")
        wk_sb = pool_const.tile([P, NC_CHUNKS * P], F32R, tag="wk")
        wv_sb = pool_const.tile([P, NC_CHUNKS * P], F32R, tag="wv")
        for w_sb, w_dram in ((wq_sb, wq), (wk_sb, wk), (wv_sb, wv)):
            # (C, 128) -> (128, 8*128): chunk cb occupies cols [128cb, 128cb+128)
            nc.sync.dma_start(
                w_sb[:].rearrange("p (cb h) -> p cb h", cb=NC_CHUNKS),
                w_dram.rearrange("(cb p) h -> p cb h", p=P))
        maskA_sb = pool_const.tile([P, 2 * TQ], F32R, tag="maskA")
        nc.sync.dma_start(maskA_sb[:], maskA[:])
        maskB_sb = pool_const.tile([P, 2 * TQ], F32R, tag="maskB")
        nc.sync.dma_start(maskB_sb[:], maskB[:])

        with tc.tile_pool(name="ps_qkv", bufs=2, space="PSUM") as ps_qkv, \
             tc.tile_pool(name="ps_s", bufs=2, space="PSUM") as ps_s, \
             tc.tile_pool(name="ps_y", bufs=2, space="PSUM") as ps_y:

            wpT_sb = None
            for b in range(B):
                # ---- load x^T chunks ----
                xc = []
                for cb in range(NC_CHUNKS):
                    xt = pool_x.tile([P, T], F32R, tag="xc")
                    nc.sync.dma_start(xt[:], xT[b][P * cb:P * (cb + 1), :])
                    xc.append(xt)

                # ---- q/k projections (transposed layout: hd x t) ----
                qT = pool_qk.tile([P, T], F32R, tag="qT")
                kT = pool_qk.tile([P, T], F32R, tag="kT")
                for w_sb, dest in ((wq_sb, qT), (wk_sb, kT)):
                    for tch in range(NQ):
                        ps = ps_qkv.tile([P, TQ], F32, tag="qkvps")
                        for cb in range(NC_CHUNKS):
                            nc.tensor.matmul(
                                ps[:],
                                w_sb[:, P * cb:P * (cb + 1)],
                                xc[cb][:, TQ * tch:TQ * (tch + 1)],
                                start=(cb == 0), stop=(cb == NC_CHUNKS - 1))
                        nc.vector.tensor_copy(
                            dest[:, TQ * tch:TQ * (tch + 1)], ps[:])

                # ---- v projection + transpose to natural layout + ones col --
                vaug = []
                for tch in range(NQ):
                    psv = ps_qkv.tile([P, TQ], F32, tag="qkvps")
                    for cb in range(NC_CHUNKS):
                        nc.tensor.matmul(
                            psv[:],
                            wv_sb[:, P * cb:P * (cb + 1)],
                            xc[cb][:, TQ * tch:TQ * (tch + 1)],
                            start=(cb == 0), stop=(cb == NC_CHUNKS - 1))
                    vtmp = pool_vt.tile([P, TQ], F32, tag="vtmp")
                    nc.vector.tensor_copy(vtmp[:], psv[:])
                    for tt in range(4):
                        pst = ps_qkv.tile([P, TQ], F32, tag="qkvps")
                        nc.tensor.transpose(
                            pst[:, 0:P], vtmp[:, P * tt:P * (tt + 1)],
                            identity[:])
                        va = pool_vaug.tile([P, 2 * (DH + 1)], F32R, tag="va")
                        nc.vector.tensor_copy(va[:, 0:DH], pst[:, 0:DH])
                        nc.vector.tensor_copy(
                            va[:, DH + 1:2 * DH + 1], pst[:, DH:2 * DH])
                        # ones columns (maskA col TQ-1 is all ones, f32r)
                        nc.vector.tensor_copy(
                            va[:, DH:DH + 1], maskA_sb[:, TQ - 1:TQ])
                        nc.vector.tensor_copy(
                            va[:, 2 * DH + 1:2 * DH + 2],
                            maskA_sb[:, TQ - 1:TQ])
                        vaug.append(va)

                if b == 0:
                    # emit the projection-weight loads here so their DMA-queue
                    # position is behind batch-0's x loads (they are only
                    # needed after the AllToAll)
                    wpT_sb = pool_const.tile([P, NC_CHUNKS * C], F32R,
                                             tag="wpt")
                    nc.sync.dma_start(
                        wpT_sb[:].rearrange("p (cb h) -> p cb h",
                                            cb=NC_CHUNKS),
                        wpT.rearrange("(cb p) h -> p cb h", p=P))

                # ---- causal attention, 2 heads row-packed ----
                for i in range(NQ):
                    psy = [ps_y.tile([DH + 1, TQ], F32, tag="psy",
                                     name=f"psy{b}_{i}_{h}")
                           for h in range(HPC)]
                    nslab = 2 * (i + 1)   # groups of 2 s-chunks
                    jmax = 4 * i + 3
                    for g in range(nslab):
                        for h in range(HPC):
                            hq = kT[DH * h:DH * (h + 1), :]
                            pss = ps_s.tile([P, 2 * TQ], F32, tag="pss")
                            for u in range(2):
                                j = 2 * g + u
                                nc.tensor.matmul(
                                    pss[:, TQ * u:TQ * (u + 1)],
                                    hq[:, TS * j:TS * (j + 1)],
                                    qT[DH * h:DH * (h + 1),
                                       TQ * i:TQ * (i + 1)],
                                    start=True, stop=True)
                            pt = pool_pt.tile([P, 2 * TQ], F32R, tag="pt")
                            nc.scalar.activation(pt[:], pss[:], EXP,
                                                 scale=float(SCALE))
                            if g == nslab - 2:
                                nc.vector.tensor_mul(pt[:], pt[:], maskA_sb[:])
                            elif g == nslab - 1:
                                nc.vector.tensor_mul(pt[:], pt[:], maskB_sb[:])
                            for u in range(2):
                                j = 2 * g + u
                                nc.tensor.matmul(
                                    psy[h][:],
                                    vaug[j][:, (DH + 1) * h:(DH + 1) * (h + 1)],
                                    pt[:, TQ * u:TQ * (u + 1)],
                                    start=(j == 0), stop=(j == jmax))
                    # ---- normalize + stage for AllToAll ----
                    for h in range(HPC):
                        rl = pool_ytn.tile([1, TQ], F32, tag="rl")
                        nc.vector.reciprocal(rl[:], psy[h][DH:DH + 1, :])
                        rlb = pool_ytn.tile([DH, TQ], F32, tag="rlb")
                        nc.gpsimd.partition_broadcast(rlb[:], rl[:])
                        ytn = pool_ytn.tile([DH, TQ], F32R, tag="ytn")
                        nc.vector.tensor_mul(ytn[:], psy[h][0:DH, :], rlb[:])
                        base = P * (NQ * b + i) + DH * h
                        nc.sync.dma_start(x_a2a[base:base + DH, :], ytn[:])

            nc.gpsimd.collective_compute(
                "AllToAll", mybir.AluOpType.bypass,
                replica_groups=[list(range(N_CORES))],
                ins=[x_a2a[:]], outs=[o_a2a[:]])

        # ---- output projection on own 512-row slice ----
        with tc.tile_pool(name="ps_proj", bufs=8, space="PSUM") as ps_proj:
            pso = [ps_proj.tile([P, TQ], F32, tag="pso", name=f"pso{m}")
                   for m in range(8)]
            for k in range(NC_CHUNKS):
                o_k = pool_o.tile([P, TQ], F32R, tag="ok")
                nc.sync.dma_start(o_k[:], o_a2a[P * k:P * (k + 1), :])
                for t in range(4):
                    for n in range(2):
                        nc.tensor.matmul(
                            pso[2 * t + n][:],
                            o_k[:, P * t:P * (t + 1)],
                            wpT_sb[:, C * k + TQ * n:C * k + TQ * (n + 1)],
                            start=(k == 0), stop=(k == NC_CHUNKS - 1))
            for t in range(4):
                for n in range(2):
                    outsb = pool_outsb.tile([P, TQ], F32, tag="outsb")
                    nc.vector.tensor_copy(outsb[:], pso[2 * t + n][:])
                    nc.sync.dma_start(
                        out[P * t:P * (t + 1), TQ * n:TQ * (n + 1)], outsb[:])

    nc.compile()
    return nc


def _make_masks():
    r = np.arange(P)[:, None]
    c = np.arange(TQ)[None, :]
    m = [(c >= r + P * mm).astype(np.float32) for mm in range(4)]
    maskA = np.concatenate([m[0], m[1]], axis=1)
    maskB = np.concatenate([m[2], m[3]], axis=1)
    return maskA, maskB


def _prep_in_maps(x, Wq, Wk, Wv, Wproj, bproj):
    x = np.asarray(x, dtype=np.float32)
    Wq = np.asarray(Wq, dtype=np.float32)
    Wk = np.asarray(Wk, dtype=np.float32)
    Wv = np.asarray(Wv, dtype=np.float32)
    Wproj = np.asarray(Wproj, dtype=np.float32)
    bproj = np.asarray(bproj, dtype=np.float32)

    xT = [np.ascontiguousarray(x[b].T) for b in range(B)]
    wpT = np.ascontiguousarray(Wproj.T)
    maskA, maskB = _make_masks()

    in_maps = []
    for c in range(N_CORES):
        h0 = HPC * c
        in_maps.append({
            "xT0": xT[0], "xT1": xT[1],
            "wq": np.ascontiguousarray(
                Wq[:, h0:h0 + HPC, :].reshape(C, HPC * DH)),
            "wk": np.ascontiguousarray(
                Wk[:, h0:h0 + HPC, :].reshape(C, HPC * DH)),
            "wv": np.ascontiguousarray(
                Wv[:, h0:h0 + HPC, :].reshape(C, HPC * DH)),
            "wpT": wpT,
            "maskA": maskA,
            "maskB": maskB,
        })
    return in_maps


def kernel(x, Wq, Wk, Wv, Wproj, bproj):
    if "nc" not in _CACHED:
        _CACHED["nc"] = _build()
    nc = _CACHED["nc"]
    in_maps = _prep_in_maps(x, Wq, Wk, Wv, Wproj, bproj)
    res = run_bass_kernel_spmd(nc, in_maps, list(range(N_CORES)))
    full = np.empty((B * T, C), dtype=np.float32)
    for c in range(N_CORES):
        full[TQ * c:TQ * (c + 1), :] = res.results[c]["out"]
    full += np.asarray(bproj, dtype=np.float32)[None, :]
    return full.reshape(B, T, C)
